# revision 26
# baseline (speedup 1.0000x reference)
"""Trainium2 Bass kernel for the CSVT point-cloud token-attention block.

Strategy (8 NeuronCores, one point cloud per core):
  tokens = (Wq^T S) diag(1/z),  S = x^T e          (never materialize xq)
  dm     = x (Wp T_P)                              (never materialize xp)
  xr     = softmax(dm) (T_P^T Wtrans)              (never materialize df)
Global BatchNorm statistics travel as tiny per-cloud sufficient statistics
(M = dmx^T dmx, u = colsum dmx, H) via one small AllGather; a dummy warm-up
collective at kernel start absorbs the first-collective staging latency.

v2 changes vs the 207us baseline:
  - The warm-up consumer (wz/wzz) is emitted right before the real
    AllGather instead of at program start; the old placement put a
    warmup-dependent op at the head of the in-order Vector queue and
    stalled the whole machine ~45us mid-kernel.
  - Point-softmax logits are computed P-major directly (288 small
    matmuls with the x-tile as stationary weights) instead of T-major +
    144 PE transposes + psum copies; z comes from one DVE reduce plus a
    ones-column matmul.
  - BN scale/bias are folded into the PE: H is rebuilt c-major, scaled
    by a per-partition, augmented with a b-row, and matched with the
    ones-row already present in the transposed dmx; phase-10 psum is
    a*xr+b directly, so the tail is relu+add only, split 3 ways across
    ACT / DVE / Pool.  Output DMA issues from the Sync queue.
  - The transposed dmx keeps its ones row (17-row slabs at partition
    bases 0/64), so no separate bias/add pass is needed.
"""
import sys

sys.path.insert(0, "/opt/trn_rl_repo")

import numpy as np
import ml_dtypes

N_CORES = 8
C = 256
T = 16
EPS = 1e-5
SHIFT = 12.0

_cache = {}


def _build(P_pad, n_cores, n_total):
    import concourse.bass as bass
    import concourse.mybir as mybir
    import concourse.tile as tile
    from concourse import bacc

    bf16 = mybir.dt.bfloat16
    f32 = mybir.dt.float32
    AF = mybir.ActivationFunctionType
    AX = mybir.AxisListType.X
    ALU = mybir.AluOpType

    assert P_pad % 6144 == 0
    NI = P_pad // 128   # 128-point tiles
    NG = NI // 8        # 8-tile groups
    NT2 = NI // 2       # transpose blocks (2 tiles each)
    NB = NT2 // 2       # dmxT block columns (2 blocks per 128 partitions)
    QN = P_pad // 6     # points per x sixth
    NXP = NI // 16      # xpg groups
    NJP = P_pad // 1024 # output chunks per k-half
    GW = 16 * n_cores   # gathered stat rows

    nc = bacc.Bacc("TRN2", target_bir_lowering=False, debug=False)

    d_xT = nc.dram_tensor("xT", [C, P_pad], bf16, kind="ExternalInput").ap()
    # xp is host-pre-tiled P-major: [128, NI, C], row p holds points i*128+p
    d_xp = nc.dram_tensor("xp", [128, NI, C], bf16, kind="ExternalInput").ap()
    d_wk = nc.dram_tensor("wk", [C, T], bf16, kind="ExternalInput").ap()
    wnames = ["wq", "wvT", "wkeT", "wqeT", "wembT", "wtT", "wpT", "wtrans"]
    d_w = {n: nc.dram_tensor(n, [C, C], bf16, kind="ExternalInput").ap() for n in wnames}
    d_gb = nc.dram_tensor("gb", [128, 4], f32, kind="ExternalInput").ap()
    d_npad = nc.dram_tensor("npadv", [1, 16], f32, kind="ExternalInput").ap()
    d_mask = nc.dram_tensor("maskpm", [128, NI], f32, kind="ExternalInput").ap()
    d_mfm = nc.dram_tensor("mfm", [16, GW], f32, kind="ExternalInput").ap()
    d_identb = nc.dram_tensor("identb", [128, 128], bf16, kind="ExternalInput").ap()
    d_identf = nc.dram_tensor("identf", [128, 128], f32, kind="ExternalInput").ap()
    d_onesrow = nc.dram_tensor("onesrow", [1, 128], f32, kind="ExternalInput").ap()
    d_onescol = nc.dram_tensor("onescol", [128, 1], f32, kind="ExternalInput").ap()
    d_yout = nc.dram_tensor("yout", [C, P_pad], bf16, kind="ExternalOutput").ap()

    xTr = d_xT.rearrange("(k p) n -> p k n", p=128)
    youtr = d_yout.rearrange("(k p) n -> p k n", p=128)

    with tile.TileContext(nc) as tc:
        with (
            tc.tile_pool(name="const", bufs=1) as const,
            tc.tile_pool(name="xc", bufs=6) as xcp,
            tc.tile_pool(name="xpp", bufs=4) as xpp,
            tc.tile_pool(name="big", bufs=1) as big,
            tc.tile_pool(name="work", bufs=1) as work,
            tc.tile_pool(name="psum", bufs=3, space="PSUM") as psum,
            tc.tile_pool(name="psbig", bufs=4, space="PSUM") as psbig,
            tc.tile_pool(name="psacc", bufs=1, space="PSUM") as psacc,
            tc.tile_pool(name="dram", bufs=1, space="DRAM") as dramp,
        ):
            # ---- warm-up collective first (absorbs CC staging latency).
            # Its result is consumed much later, right before the real
            # AllGather, so nothing here blocks the main pipeline.
            ws = const.tile([16, 16], f32)
            nc.vector.memset(ws, 1.0)
            wcc_in = dramp.tile([16, 16], f32)
            wcc_out = dramp.tile([GW, 16], f32)
            nc.sync.dma_start(wcc_in, ws)
            nc.gpsimd.collective_compute(
                "AllGather", ALU.bypass,
                replica_groups=[list(range(n_cores))],
                ins=[wcc_in.opt()], outs=[wcc_out.opt()],
            )

            # ---- consts + weights FIRST on the Sync queue (~1.4 MB, ~4us).
            # NOT on the GpSimd queue: gpsimd-issued DMAs share a ring with
            # the collective and would sit behind the warm-up barrier. ----
            wk_sb = const.tile([128, 2, T], bf16)
            nc.sync.dma_start(wk_sb, d_wk.rearrange("(k p) t -> p k t", p=128))
            identb = const.tile([128, 128], bf16)
            nc.sync.dma_start(identb, d_identb)
            identf = const.tile([128, 128], f32)
            nc.sync.dma_start(identf, d_identf)
            onesrow = const.tile([1, 128], f32)
            nc.sync.dma_start(onesrow, d_onesrow)
            onescol = const.tile([128, 1], f32)
            nc.sync.dma_start(onescol, d_onescol)
            npad_sb = const.tile([1, 16], f32)
            nc.sync.dma_start(npad_sb, d_npad)
            mask_sb = const.tile([128, NI], f32)
            nc.sync.dma_start(mask_sb, d_mask)
            gb_sb = const.tile([128, 4], f32)
            nc.sync.dma_start(gb_sb, d_gb)
            mfm_sb = const.tile([16, GW], f32)
            nc.sync.dma_start(mfm_sb, d_mfm)
            w_sb = {}
            for n in wnames:
                w_sb[n] = const.tile([128, 2, C], bf16, tag=f"w_{n}", name=f"w_{n}")
                nc.sync.dma_start(w_sb[n], d_w[n].rearrange("(k p) c -> p k c", p=128))

            # ---- x stream on the Sync DMA queue: 6 sixths, 9 xpg ----
            xs = []
            for q in range(6):
                t = xcp.tile([128, 2, QN], bf16, tag="xc", name="xc")
                nc.sync.dma_start(t, xTr[:, :, q * QN:(q + 1) * QN])
                xs.append(t)
            xpgs = []
            for e in range(NXP):
                xpg = xpp.tile([128, 16, C], bf16, tag="xpg", name="xpg")
                nc.sync.dma_start(xpg, d_xp[:, e * 16:(e + 1) * 16, :])
                xpgs.append(xpg)
            epsv = const.tile([128, 1], f32)
            nc.vector.memset(epsv, EPS)
            shiftv = const.tile([128, 1], f32)
            nc.vector.memset(shiftv, -SHIFT)

            # ---- phase E: e = exp(x Wk - SHIFT), P-major [128, NI, 16] ----
            e_sb = big.tile([128, NI, T], bf16, tag="bigA", name="e_sb")
            for g in range(NG):
                pe = psum.tile([128, 8, 16], f32, tag="ps")
                for i8 in range(8):
                    i = g * 8 + i8
                    si, lc = divmod(i, NI // 6)
                    for k in range(2):
                        nc.tensor.matmul(pe[:, i8, :],
                                         xs[si][:, k, lc * 128:(lc + 1) * 128],
                                         wk_sb[:, k, :],
                                         start=(k == 0), stop=(k == 1))
                nc.scalar.activation(e_sb[:, g * 8:(g + 1) * 8, :], pe,
                                     AF.Exp, bias=shiftv)

            # ---- phase S: S^T = e^T x, accumulated over point tiles ----
            pS = psacc.tile([16, 256], f32, tag="acc")
            for e in range(NXP):
                for s in range(16):
                    i = e * 16 + s
                    nc.tensor.matmul(pS, e_sb[:, i, :], xpgs[e][:, s, :],
                                     start=(i == 0), stop=(i == NI - 1))
            sT = work.tile([16, 256], bf16, tag="sT")
            nc.scalar.copy(sT, pS)

            # ---- z = colsum e (pad-corrected), as a [1,16] row ----
            zpart = work.tile([128, 16], f32, tag="zpart")
            e_swp = bass.AP(e_sb.tensor, e_sb.offset,
                            [e_sb.ap[0], e_sb.ap[2], e_sb.ap[1]])
            nc.vector.reduce_sum(zpart, e_swp, axis=AX)
            zps = psum.tile([1, 16], f32, tag="ps")
            nc.tensor.matmul(zps, onescol, zpart)
            zrow = work.tile([1, 16], f32, tag="zrow")
            nc.scalar.copy(zrow, zps)
            zc = work.tile([1, 16], f32, tag="zc")
            nc.vector.tensor_sub(zc, zrow, npad_sb)  # npadv pre-scaled by e^-SHIFT
            zinv = work.tile([1, 16], f32, tag="zinv")
            nc.vector.reciprocal(zinv, zc)

            # ---- phase 5: tokens = (Wq^T S) diag(zinv) ----
            scm = work.tile([128, 2, 16], bf16, tag="scm")
            for k in range(2):
                pt = psum.tile([128, 16], bf16, tag="ps")
                nc.tensor.transpose(pt, sT[:, k * 128:(k + 1) * 128], identb[0:16, 0:16])
                nc.scalar.copy(scm[:, k, :], pt)
            pzb = psum.tile([128, 16], f32, tag="ps")
            nc.tensor.matmul(pzb, onesrow, zinv)
            zb = work.tile([128, 16], f32, tag="zb")
            nc.scalar.copy(zb, pzb)
            tok = work.tile([128, 2, 16], bf16, tag="tok")
            for ko in range(2):
                ptok = psum.tile([128, 16], f32, tag="ps")
                for ki in range(2):
                    nc.tensor.matmul(ptok, w_sb["wq"][:, ki, ko * 128:(ko + 1) * 128],
                                     scm[:, ki, :], start=(ki == 0), stop=(ki == 1))
                nc.vector.tensor_mul(tok[:, ko, :], ptok, zb)

            # ---- phase 5b: token self-attention ----
            def cmajor_mm(wname, rhs_tile, out_name, post=None):
                out = work.tile([128, 2, 16], bf16, tag=out_name, name=out_name)
                for ko in range(2):
                    p = psum.tile([128, 16], f32, tag="ps")
                    for ki in range(2):
                        nc.tensor.matmul(p, w_sb[wname][:, ki, ko * 128:(ko + 1) * 128],
                                         rhs_tile[:, ki, :], start=(ki == 0), stop=(ki == 1))
                    if post is None:
                        nc.scalar.copy(out[:, ko, :], p)
                    else:
                        post(out, ko, p)
                return out

            keys = cmajor_mm("wkeT", tok, "keys")
            qrs = cmajor_mm("wqeT", tok, "qrs")
            pv = psum.tile([16, 256], f32, tag="ps")
            for k in range(2):
                nc.tensor.matmul(pv, tok[:, k, :], w_sb["wvT"][:, k, :],
                                 start=(k == 0), stop=(k == 1))
            valsT = work.tile([16, 256], bf16, tag="valsT")
            nc.scalar.copy(valsT, pv)
            plg = psum.tile([16, 16], f32, tag="ps")
            for k in range(2):
                nc.tensor.matmul(plg, keys[:, k, :], qrs[:, k, :],
                                 start=(k == 0), stop=(k == 1))
            nmx2 = work.tile([16, 1], f32, tag="nmx2")
            nc.vector.reduce_max(nmx2, plg, axis=AX, negate=True)
            vtf = work.tile([16, 16], f32, tag="vtf")
            z2 = work.tile([16, 1], f32, tag="z2")
            nc.scalar.activation(vtf, plg, AF.Exp, bias=nmx2, accum_out=z2)
            z2i = work.tile([16, 1], f32, tag="z2i")
            nc.vector.reciprocal(z2i, z2)
            vt = work.tile([16, 16], bf16, tag="vt")
            nc.vector.tensor_scalar_mul(vt, vtf, z2i)
            pvtT = psum.tile([16, 16], bf16, tag="ps")
            nc.tensor.transpose(pvtT, vt, identb[0:16, 0:16])
            vtT = work.tile([16, 16], bf16, tag="vtT")
            nc.scalar.copy(vtT, pvtT)
            tm = work.tile([128, 2, 16], bf16, tag="tm")
            for ko in range(2):
                ptm = psum.tile([128, 16], f32, tag="ps")
                nc.tensor.matmul(ptm, valsT[:, ko * 128:(ko + 1) * 128], vtT)
                nc.scalar.copy(tm[:, ko, :], ptm)

            def add_tok(out, ko, p):
                nc.vector.tensor_add(out[:, ko, :], p, tok[:, ko, :])

            tout = cmajor_mm("wembT", tm, "tout", post=add_tok)
            tp = cmajor_mm("wtT", tout, "tp")
            g_sb = cmajor_mm("wpT", tp, "g_sb")
            # H in T-major [16,256] for the stats AllGather
            ph = psum.tile([16, 256], f32, tag="ps")
            for k in range(2):
                nc.tensor.matmul(ph, tp[:, k, :], w_sb["wtrans"][:, k, :],
                                 start=(k == 0), stop=(k == 1))
            h32 = work.tile([16, 256], f32, tag="h32")
            nc.vector.tensor_copy(h32, ph)
            # H in c-major [128, 2, 16] for the phase-10 lhsT (scaled later)
            hc_sb = cmajor_mm("wtrans", tp, "hc_sb")

            # ---- phase 6+7 fused: dm = x G, exp, P-major [128, NI, 64] ----
            dmxw = big.tile([128, NI, 64], bf16, tag="bigB", name="dmxw")
            nc.vector.memset(dmxw[:, :, 17:64], 0.0)
            nc.vector.memset(dmxw[:, :, 16:17], 1.0)
            # dmxw slot order interleaves the two point-cloud halves
            # (slot 2q = tile q, slot 2q+1 = tile NT2+q) so the phase-7.5
            # transposes read contiguous [128, 2, 64] blocks.
            for g in range(NG):
                pdm = psum.tile([128, 8, 16], f32, tag="ps")
                for i8 in range(8):
                    i = g * 8 + i8
                    qt, lc = divmod(i, NI // 6)
                    for k in range(2):
                        nc.tensor.matmul(pdm[:, i8, :],
                                         xs[qt][:, k, lc * 128:(lc + 1) * 128],
                                         g_sb[:, k, :], start=(k == 0), stop=(k == 1))
                s0 = 16 * g if g < NG // 2 else 16 * g - NI + 1
                db = dmxw[:, s0, 0:16]
                dst = bass.AP(db.tensor, db.offset, [db.ap[0], [128, 8], [1, 16]])
                nc.scalar.activation(dst, pdm, AF.Exp, bias=shiftv)
            zd = work.tile([128, NI], f32, tag="zd")
            nc.vector.reduce_sum(zd, dmxw[:, :, 0:16], axis=AX)
            nc.vector.reciprocal(zd, zd)
            nc.vector.tensor_mul(zd, zd, mask_sb)
            zdb = bass.AP(zd.tensor, zd.offset, list(zd.ap) + [[0, T]])
            nc.vector.tensor_mul(dmxw[:, :, 0:16], dmxw[:, :, 0:16], zdb)

            # ---- phase 8: M = dmx^T dmx (+ u via ones column), AllGather ----
            pmu = psacc.tile([16, 17], f32, tag="acc", name="pmu")
            for i in range(NI):
                nc.tensor.matmul(pmu, dmxw[:, i, 0:16], dmxw[:, i, 0:17],
                                 start=(i == 0), stop=(i == NI - 1))
            # consume the warm-up collective on the GpSimd queue ONLY —
            # the tile scheduler reorders within engine queues, and a
            # warmup-dependent Vector op would block the whole Vector
            # queue until the warm-up AllGather lands (~90us).
            wz = work.tile([16, 1], f32, tag="wz")
            nc.gpsimd.dma_start(wz, wcc_out[0:16, 0:1])
            wzz = work.tile([16, 1], f32, tag="wzz")
            nc.gpsimd.tensor_scalar_mul(wzz, wz, 0.0)
            ccin = work.tile([16, GW + 257], f32, tag="ccin")
            M17 = work.tile([16, 17], f32, tag="M17")
            nc.scalar.copy(M17, pmu)
            mrep = bass.AP(M17.tensor, M17.offset,
                           [M17.ap[0], [0, n_cores], [1, 16]])
            nc.vector.tensor_mul(ccin[:, 0:GW], mrep, mfm_sb)
            nc.gpsimd.tensor_add(ccin[:, GW:GW + 1], M17[:, 16:17], wzz)
            nc.vector.tensor_copy(ccin[:, GW + 1:GW + 257], h32)
            ccd_in = dramp.tile([16, GW + 257], f32)
            ccd_out = dramp.tile([GW, GW + 257], f32)
            nc.sync.dma_start(ccd_in, ccin)
            nc.gpsimd.collective_compute(
                "AllGather", ALU.bypass,
                replica_groups=[list(range(n_cores))],
                ins=[ccd_in.opt()], outs=[ccd_out.opt()],
            )

            # ---- phase 7.5 (overlaps the gather): transpose dmx ----
            # dmxT block q holds tile q at partitions 0:17 (16 dm rows +
            # the ones row) and tile NT2+q at partitions 64:81, so each
            # phase-10 half covers a CONTIGUOUS 512-point range and the
            # tail adds / DMA need no strided operands.
            dmxT = big.tile([128, NT2, 128], bf16, tag="bigA", name="dmxT")
            for q in range(NT2):
                ptd = psum.tile([128, 128], bf16, tag="ps")
                nc.tensor.transpose(ptd, dmxw[:, 2 * q:2 * q + 2, :], identb)
                if q % 2 == 0:
                    nc.scalar.copy(dmxT[:, q, :], ptd)
                else:
                    nc.vector.tensor_copy(dmxT[:, q, :], ptd)

            gth = work.tile([GW, GW + 257], f32, tag="gth")
            nc.sync.dma_start(gth, ccd_out)

            # ---- phase 9: global BN stats from gathered {M, u, H} ----
            pY = psum.tile([GW, 256], f32, tag="ps")
            nc.tensor.matmul(pY, gth[:, 0:GW], gth[:, GW + 1:GW + 257])
            yh = work.tile([GW, 256], f32, tag="yh")
            nc.vector.tensor_mul(yh, pY, gth[:, GW + 1:GW + 257])
            pq = psum.tile([1, 256], f32, tag="ps")
            nc.tensor.matmul(pq, onescol[0:GW, :], yh)
            ps_ = psum.tile([1, 256], f32, tag="ps")
            nc.tensor.matmul(ps_, gth[:, GW:GW + 1], gth[:, GW + 1:GW + 257])
            sq = work.tile([1, 512], f32, tag="sq")
            nc.scalar.copy(sq[:, 0:256], ps_)
            nc.scalar.copy(sq[:, 256:512], pq)
            sqT = work.tile([128, 4], f32, tag="sqT")
            for h in range(4):
                pt = psum.tile([128, 1], f32, tag="ps")
                nc.tensor.transpose(pt, sq[:, h * 128:(h + 1) * 128], identf[0:1, 0:1])
                nc.scalar.copy(sqT[:, h:h + 1], pt)
            mean = work.tile([128, 2], f32, tag="mean")
            nc.vector.tensor_scalar_mul(mean, sqT[:, 0:2], 1.0 / n_total)
            ex2 = work.tile([128, 2], f32, tag="ex2")
            nc.vector.tensor_scalar_mul(ex2, sqT[:, 2:4], 1.0 / n_total)
            mm2 = work.tile([128, 2], f32, tag="mm2")
            nc.vector.tensor_mul(mm2, mean, mean)
            var = work.tile([128, 2], f32, tag="var")
            nc.vector.tensor_sub(var, ex2, mm2)
            sd = work.tile([128, 2], f32, tag="sd")
            nc.scalar.activation(sd, var, AF.Sqrt, bias=epsv)
            rstd = work.tile([128, 2], f32, tag="rstd")
            nc.vector.reciprocal(rstd, sd)
            a_sb = work.tile([128, 2], f32, tag="a_sb")
            nc.vector.tensor_mul(a_sb, gb_sb[:, 0:2], rstd)
            am = work.tile([128, 2], f32, tag="am")
            nc.vector.tensor_mul(am, a_sb, mean)
            b_sb = work.tile([128, 2], f32, tag="b_sb")
            nc.vector.tensor_sub(b_sb, gb_sb[:, 2:4], am)

            # ---- build the augmented lhsT: h2 rows = [H^T diag(a); b] ----
            hcb = work.tile([128, 2, 17], bf16, tag="hcb")
            for k in range(2):
                nc.vector.tensor_scalar_mul(hcb[:, k, 0:16], hc_sb[:, k, :],
                                            a_sb[:, k:k + 1])
                nc.vector.tensor_copy(hcb[:, k, 16:17], b_sb[:, k:k + 1])
            h2 = work.tile([128, 2, 128], bf16, tag="h2")
            for k in range(2):
                pth = psum.tile([17, 128], bf16, tag="ps")
                nc.tensor.transpose(pth, hcb[:, k, :], identb)
                nc.scalar.copy(h2[0:17, k, :], pth)
                nc.scalar.copy(h2[64:81, k, :], pth)

            # ---- phase 10: psum = a*xr + b (via 17-row matmuls), then
            # relu+residual split across ACT / DVE / Pool, DMA on Sync ----
            # Half-granular pipeline: each (jp,k,i2) half is one matmul
            # into its own 1-bank psum (bufs=4), one relu (psum->sbuf),
            # one residual add.  With the (q, NT2+q) transpose pairing
            # every half is a contiguous 512-point range, so relu, add,
            # and DMA all run on unit-stride operands.  Relus: ACT 48 /
            # DVE 24 (Pool cannot read PSUM); adds: DVE 40 / Pool 32.
            for jp in range(NJP):
                lo = (jp % 6) * 512
                ybuf = work.tile([128, 2, 2, 512], bf16, tag="ybuf", bufs=3)
                for k in range(2):
                    for i2 in range(2):
                        h = 2 * (2 * jp + k) + i2
                        base = 64 * i2
                        qt = jp // 6 + 3 * i2
                        pxh = psbig.tile([128, 512], f32, tag="pxr")
                        nc.tensor.matmul(pxh,
                                         h2[base:base + 17, k, :],
                                         dmxT[base:base + 17, 4 * jp:4 * jp + 4, :])
                        tt = work.tile([128, 512], bf16, tag="tt", bufs=6)
                        xw = xs[qt][:, k, lo:lo + 512]
                        if h % 3 == 2:
                            nc.vector.tensor_relu(tt, pxh)
                        else:
                            nc.scalar.activation(tt, pxh, AF.Relu)
                        if h % 9 < 4:
                            nc.gpsimd.tensor_add(ybuf[:, k, i2, :], tt, xw)
                        else:
                            nc.vector.tensor_add(ybuf[:, k, i2, :], tt, xw)
                for i2 in range(2):
                    off = i2 * NT2 * 128 + 512 * jp
                    nc.sync.dma_start(youtr[:, :, off:off + 512],
                                      ybuf[:, :, i2, :])

    nc.compile()
    return nc


def _prep_core(xc, P_pad, b, n_cores):
    bf = ml_dtypes.bfloat16
    cnt = xc.shape[0]
    NI = P_pad // 128
    xT = np.zeros((C, P_pad), dtype=bf)
    xT[:, :cnt] = xc.T.astype(bf)
    # P-major tiled layout [128, NI, C]: row p holds points i*128+p
    xp = np.zeros((NI * 128, C), dtype=bf)
    xp[:cnt] = xc.astype(bf)
    xp = np.ascontiguousarray(xp.reshape(NI, 128, C).transpose(1, 0, 2))
    idx = np.arange(P_pad).reshape(NI, 128).T  # [p, i] -> point index
    mask = (idx < cnt).astype(np.float32)
    # dmxw slot order: slot 2q = tile q, slot 2q+1 = tile NI//2+q
    perm = np.empty(NI, dtype=np.int64)
    perm[0::2] = np.arange(NI // 2)
    perm[1::2] = np.arange(NI // 2) + NI // 2
    mask = np.ascontiguousarray(mask[:, perm])
    mfm = np.zeros((16, 16 * n_cores), dtype=np.float32)
    mfm[:, b * 16:(b + 1) * 16] = 1.0
    npadv = np.full((1, 16), float(P_pad - cnt) * np.exp(-SHIFT), dtype=np.float32)
    return {"xT": xT, "xp": xp, "maskpm": mask, "mfm": mfm, "npadv": npadv}


def make_in_maps(x_f, counts, offs, P_pad, n_cores, Wq, Wk, Wp, Wv, Wke, Wqe,
                 Wemb, Wt, Wtrans, bn_gamma, bn_beta):
    bf = ml_dtypes.bfloat16
    g2 = np.asarray(bn_gamma, np.float32).reshape(2, 128).T
    b2 = np.asarray(bn_beta, np.float32).reshape(2, 128).T
    shared = {
        "wk": np.ascontiguousarray(Wk).astype(bf),
        "wq": np.ascontiguousarray(Wq).astype(bf),
        "wvT": np.ascontiguousarray(np.asarray(Wv).T).astype(bf),
        "wkeT": np.ascontiguousarray(np.asarray(Wke).T).astype(bf),
        "wqeT": np.ascontiguousarray(np.asarray(Wqe).T).astype(bf),
        "wembT": np.ascontiguousarray(np.asarray(Wemb).T).astype(bf),
        "wtT": np.ascontiguousarray(np.asarray(Wt).T).astype(bf),
        "wpT": np.ascontiguousarray(np.asarray(Wp).T).astype(bf),
        "wtrans": np.ascontiguousarray(Wtrans).astype(bf),
        "gb": np.concatenate([g2, b2], axis=1),
        "identb": np.eye(128, dtype=bf),
        "identf": np.eye(128, dtype=np.float32),
        "onesrow": np.ones((1, 128), dtype=np.float32),
        "onescol": np.ones((128, 1), dtype=np.float32),
    }
    in_maps = []
    for b in range(n_cores):
        m = _prep_core(x_f[offs[b]:offs[b + 1]], P_pad, b, n_cores)
        m.update(shared)
        in_maps.append(m)
    return in_maps


def kernel(x_f, batch_ids, Wq, Wk, Wp, Wv, Wke, Wqe, Wemb, Wt, Wtrans,
           bn_gamma, bn_beta):
    from concourse.bass_utils import run_bass_kernel_spmd

    x_f = np.asarray(x_f, dtype=np.float32)
    batch_ids = np.asarray(batch_ids)
    n_total = x_f.shape[0]
    counts = np.bincount(batch_ids, minlength=N_CORES)
    offs = np.concatenate([[0], np.cumsum(counts)])
    P_pad = int(-(-counts.max() // 6144) * 6144)

    key = (P_pad, N_CORES, n_total)
    if key not in _cache:
        _cache[key] = _build(P_pad, N_CORES, n_total)
    nc = _cache[key]

    in_maps = make_in_maps(x_f, counts, offs, P_pad, N_CORES, Wq, Wk, Wp, Wv,
                           Wke, Wqe, Wemb, Wt, Wtrans, bn_gamma, bn_beta)
    res = run_bass_kernel_spmd(nc, in_maps, list(range(N_CORES)))

    out = np.empty((n_total, C), dtype=np.float32)
    for b in range(N_CORES):
        yT = np.asarray(res.results[b]["yout"])  # [C, P_pad] bf16
        out[offs[b]:offs[b + 1]] = yT[:, :counts[b]].T.astype(np.float32)
    return out


# revision 27
# speedup vs baseline: 1.0553x; 1.0553x over previous
"""Trainium2 Bass kernel for the CSVT point-cloud token-attention block.

Strategy (8 NeuronCores, one point cloud per core):
  tokens = (Wq^T S) diag(1/z),  S = x^T e          (never materialize xq)
  dm     = x (Wp T_P)                              (never materialize xp)
  xr     = softmax(dm) (T_P^T Wtrans)              (never materialize df)
Global BatchNorm statistics travel as tiny per-cloud sufficient statistics
(M = dmx^T dmx, u = colsum dmx, H) via one small AllGather; a dummy warm-up
collective at kernel start absorbs the first-collective staging latency.

v2 changes vs the 207us baseline:
  - The warm-up consumer (wz/wzz) is emitted right before the real
    AllGather instead of at program start; the old placement put a
    warmup-dependent op at the head of the in-order Vector queue and
    stalled the whole machine ~45us mid-kernel.
  - Point-softmax logits are computed P-major directly (288 small
    matmuls with the x-tile as stationary weights) instead of T-major +
    144 PE transposes + psum copies; z comes from one DVE reduce plus a
    ones-column matmul.
  - BN scale/bias are folded into the PE: H is rebuilt c-major, scaled
    by a per-partition, augmented with a b-row, and matched with the
    ones-row already present in the transposed dmx; phase-10 psum is
    a*xr+b directly, so the tail is relu+add only, split 3 ways across
    ACT / DVE / Pool.  Output DMA issues from the Sync queue.
  - The transposed dmx keeps its ones row (17-row slabs at partition
    bases 0/64), so no separate bias/add pass is needed.
"""
import sys

sys.path.insert(0, "/opt/trn_rl_repo")

import numpy as np
import ml_dtypes

N_CORES = 8
C = 256
T = 16
EPS = 1e-5
SHIFT = 12.0

_cache = {}


def _build(P_pad, n_cores, n_total):
    import concourse.bass as bass
    import concourse.mybir as mybir
    import concourse.tile as tile
    from concourse import bacc

    bf16 = mybir.dt.bfloat16
    f32 = mybir.dt.float32
    AF = mybir.ActivationFunctionType
    AX = mybir.AxisListType.X
    ALU = mybir.AluOpType

    assert P_pad % 6144 == 0
    NI = P_pad // 128   # 128-point tiles
    NG = NI // 8        # 8-tile groups
    NT2 = NI // 2       # transpose blocks (2 tiles each)
    NB = NT2 // 2       # dmxT block columns (2 blocks per 128 partitions)
    QN = P_pad // 6     # points per x sixth
    NXP = NI // 16      # xpg groups
    NJP = P_pad // 1024 # output chunks per k-half
    GW = 16 * n_cores   # gathered stat rows

    nc = bacc.Bacc("TRN2", target_bir_lowering=False, debug=False)

    d_xT = nc.dram_tensor("xT", [C, P_pad], bf16, kind="ExternalInput").ap()
    # xp is host-pre-tiled P-major: [128, NI, C], row p holds points i*128+p
    d_xp = nc.dram_tensor("xp", [128, NI, C], bf16, kind="ExternalInput").ap()
    d_wk = nc.dram_tensor("wk", [C, T], bf16, kind="ExternalInput").ap()
    wnames = ["wq", "wvT", "wkeT", "wqeT", "wembT", "wtT", "wpT", "wtrans"]
    d_w = {n: nc.dram_tensor(n, [C, C], bf16, kind="ExternalInput").ap() for n in wnames}
    d_gb = nc.dram_tensor("gb", [128, 4], f32, kind="ExternalInput").ap()
    d_npad = nc.dram_tensor("npadv", [1, 16], f32, kind="ExternalInput").ap()
    d_mask = nc.dram_tensor("maskpm", [128, NI], f32, kind="ExternalInput").ap()
    d_mfm = nc.dram_tensor("mfm", [16, GW], f32, kind="ExternalInput").ap()
    d_identb = nc.dram_tensor("identb", [128, 128], bf16, kind="ExternalInput").ap()
    d_identf = nc.dram_tensor("identf", [128, 128], f32, kind="ExternalInput").ap()
    d_onesrow = nc.dram_tensor("onesrow", [1, 128], f32, kind="ExternalInput").ap()
    d_onescol = nc.dram_tensor("onescol", [128, 1], f32, kind="ExternalInput").ap()
    d_yout = nc.dram_tensor("yout", [C, P_pad], bf16, kind="ExternalOutput").ap()

    xTr = d_xT.rearrange("(k p) n -> p k n", p=128)
    youtr = d_yout.rearrange("(k p) n -> p k n", p=128)

    with tile.TileContext(nc) as tc:
        with (
            tc.tile_pool(name="const", bufs=1) as const,
            tc.tile_pool(name="xc", bufs=6) as xcp,
            tc.tile_pool(name="xpp", bufs=4) as xpp,
            tc.tile_pool(name="big", bufs=1) as big,
            tc.tile_pool(name="work", bufs=1) as work,
            tc.tile_pool(name="psum", bufs=3, space="PSUM") as psum,
            tc.tile_pool(name="psbig", bufs=4, space="PSUM") as psbig,
            tc.tile_pool(name="psacc", bufs=1, space="PSUM") as psacc,
            tc.tile_pool(name="dram", bufs=1, space="DRAM") as dramp,
        ):
            # ---- warm-up collective first (absorbs CC staging latency).
            # Its result is consumed much later, right before the real
            # AllGather, so nothing here blocks the main pipeline.
            ws = const.tile([16, 16], f32)
            nc.vector.memset(ws, 1.0)
            wcc_in = dramp.tile([16, 16], f32)
            wcc_out = dramp.tile([GW, 16], f32)
            nc.sync.dma_start(wcc_in, ws)
            nc.gpsimd.collective_compute(
                "AllGather", ALU.bypass,
                replica_groups=[list(range(n_cores))],
                ins=[wcc_in.opt()], outs=[wcc_out.opt()],
            )

            # ---- consts + weights FIRST on the Sync queue (~1.4 MB, ~4us).
            # NOT on the GpSimd queue: gpsimd-issued DMAs share a ring with
            # the collective and would sit behind the warm-up barrier. ----
            wk_sb = const.tile([128, 2, T], bf16)
            nc.sync.dma_start(wk_sb, d_wk.rearrange("(k p) t -> p k t", p=128))
            identb = const.tile([128, 128], bf16)
            nc.sync.dma_start(identb, d_identb)
            identf = const.tile([128, 128], f32)
            nc.sync.dma_start(identf, d_identf)
            onesrow = const.tile([1, 128], f32)
            nc.sync.dma_start(onesrow, d_onesrow)
            onescol = const.tile([128, 1], f32)
            nc.sync.dma_start(onescol, d_onescol)
            npad_sb = const.tile([1, 16], f32)
            nc.sync.dma_start(npad_sb, d_npad)
            mask_sb = const.tile([128, NI], f32)
            nc.sync.dma_start(mask_sb, d_mask)
            gb_sb = const.tile([128, 4], f32)
            nc.sync.dma_start(gb_sb, d_gb)
            mfm_sb = const.tile([16, GW], f32)
            nc.sync.dma_start(mfm_sb, d_mfm)
            w_sb = {}
            for n in wnames:
                w_sb[n] = const.tile([128, 2, C], bf16, tag=f"w_{n}", name=f"w_{n}")
                nc.sync.dma_start(w_sb[n], d_w[n].rearrange("(k p) c -> p k c", p=128))

            # ---- x stream on the Sync DMA queue: 6 sixths, 9 xpg ----
            xs = []
            for q in range(6):
                t = xcp.tile([128, 2, QN], bf16, tag="xc", name="xc")
                nc.sync.dma_start(t, xTr[:, :, q * QN:(q + 1) * QN])
                xs.append(t)
            xpgs = []
            for e in range(NXP):
                xpg = xpp.tile([128, 16, C], bf16, tag="xpg", name="xpg")
                nc.sync.dma_start(xpg, d_xp[:, e * 16:(e + 1) * 16, :])
                xpgs.append(xpg)
            epsv = const.tile([128, 1], f32)
            nc.vector.memset(epsv, EPS)
            shiftv = const.tile([128, 1], f32)
            nc.vector.memset(shiftv, -SHIFT)

            # ---- phase E: e = exp(x Wk - SHIFT), P-major [128, NI, 16] ----
            e_sb = big.tile([128, NI, T], bf16, tag="bigA", name="e_sb")
            for g in range(NG):
                pe = psum.tile([128, 8, 16], f32, tag="ps")
                for i8 in range(8):
                    i = g * 8 + i8
                    si, lc = divmod(i, NI // 6)
                    for k in range(2):
                        nc.tensor.matmul(pe[:, i8, :],
                                         xs[si][:, k, lc * 128:(lc + 1) * 128],
                                         wk_sb[:, k, :],
                                         start=(k == 0), stop=(k == 1))
                nc.scalar.activation(e_sb[:, g * 8:(g + 1) * 8, :], pe,
                                     AF.Exp, bias=shiftv)

            # ---- phase S: S^T = e^T x, accumulated over point tiles ----
            pS = psacc.tile([16, 256], f32, tag="acc")
            for e in range(NXP):
                for s in range(16):
                    i = e * 16 + s
                    nc.tensor.matmul(pS, e_sb[:, i, :], xpgs[e][:, s, :],
                                     start=(i == 0), stop=(i == NI - 1))
            sT = work.tile([16, 256], bf16, tag="sT")
            nc.scalar.copy(sT, pS)

            # ---- z = colsum e (pad-corrected), as a [1,16] row ----
            zpart = work.tile([128, 16], f32, tag="zpart")
            e_swp = bass.AP(e_sb.tensor, e_sb.offset,
                            [e_sb.ap[0], e_sb.ap[2], e_sb.ap[1]])
            nc.vector.reduce_sum(zpart, e_swp, axis=AX)
            zps = psum.tile([1, 16], f32, tag="ps")
            nc.tensor.matmul(zps, onescol, zpart)
            zrow = work.tile([1, 16], f32, tag="zrow")
            nc.scalar.copy(zrow, zps)
            zc = work.tile([1, 16], f32, tag="zc")
            nc.vector.tensor_sub(zc, zrow, npad_sb)  # npadv pre-scaled by e^-SHIFT
            zinv = work.tile([1, 16], f32, tag="zinv")
            nc.vector.reciprocal(zinv, zc)

            # ---- phase 5: tokens = (Wq^T S) diag(zinv) ----
            scm = work.tile([128, 2, 16], bf16, tag="scm")
            for k in range(2):
                pt = psum.tile([128, 16], bf16, tag="ps")
                nc.tensor.transpose(pt, sT[:, k * 128:(k + 1) * 128], identb[0:16, 0:16])
                nc.scalar.copy(scm[:, k, :], pt)
            pzb = psum.tile([128, 16], f32, tag="ps")
            nc.tensor.matmul(pzb, onesrow, zinv)
            zb = work.tile([128, 16], f32, tag="zb")
            nc.scalar.copy(zb, pzb)
            tok = work.tile([128, 2, 16], bf16, tag="tok")
            for ko in range(2):
                ptok = psum.tile([128, 16], f32, tag="ps")
                for ki in range(2):
                    nc.tensor.matmul(ptok, w_sb["wq"][:, ki, ko * 128:(ko + 1) * 128],
                                     scm[:, ki, :], start=(ki == 0), stop=(ki == 1))
                nc.vector.tensor_mul(tok[:, ko, :], ptok, zb)

            # ---- phase 5b: token self-attention ----
            def cmajor_mm(wname, rhs_tile, out_name, post=None):
                out = work.tile([128, 2, 16], bf16, tag=out_name, name=out_name)
                for ko in range(2):
                    p = psum.tile([128, 16], f32, tag="ps")
                    for ki in range(2):
                        nc.tensor.matmul(p, w_sb[wname][:, ki, ko * 128:(ko + 1) * 128],
                                         rhs_tile[:, ki, :], start=(ki == 0), stop=(ki == 1))
                    if post is None:
                        nc.scalar.copy(out[:, ko, :], p)
                    else:
                        post(out, ko, p)
                return out

            keys = cmajor_mm("wkeT", tok, "keys")
            qrs = cmajor_mm("wqeT", tok, "qrs")
            pv = psum.tile([16, 256], f32, tag="ps")
            for k in range(2):
                nc.tensor.matmul(pv, tok[:, k, :], w_sb["wvT"][:, k, :],
                                 start=(k == 0), stop=(k == 1))
            valsT = work.tile([16, 256], bf16, tag="valsT")
            nc.scalar.copy(valsT, pv)
            plg = psum.tile([16, 16], f32, tag="ps")
            for k in range(2):
                nc.tensor.matmul(plg, keys[:, k, :], qrs[:, k, :],
                                 start=(k == 0), stop=(k == 1))
            nmx2 = work.tile([16, 1], f32, tag="nmx2")
            nc.vector.reduce_max(nmx2, plg, axis=AX, negate=True)
            vtf = work.tile([16, 16], f32, tag="vtf")
            z2 = work.tile([16, 1], f32, tag="z2")
            nc.scalar.activation(vtf, plg, AF.Exp, bias=nmx2, accum_out=z2)
            z2i = work.tile([16, 1], f32, tag="z2i")
            nc.vector.reciprocal(z2i, z2)
            vt = work.tile([16, 16], bf16, tag="vt")
            nc.vector.tensor_scalar_mul(vt, vtf, z2i)
            pvtT = psum.tile([16, 16], bf16, tag="ps")
            nc.tensor.transpose(pvtT, vt, identb[0:16, 0:16])
            vtT = work.tile([16, 16], bf16, tag="vtT")
            nc.scalar.copy(vtT, pvtT)
            tm = work.tile([128, 2, 16], bf16, tag="tm")
            for ko in range(2):
                ptm = psum.tile([128, 16], f32, tag="ps")
                nc.tensor.matmul(ptm, valsT[:, ko * 128:(ko + 1) * 128], vtT)
                nc.scalar.copy(tm[:, ko, :], ptm)

            def add_tok(out, ko, p):
                nc.vector.tensor_add(out[:, ko, :], p, tok[:, ko, :])

            tout = cmajor_mm("wembT", tm, "tout", post=add_tok)
            tp = cmajor_mm("wtT", tout, "tp")
            g_sb = cmajor_mm("wpT", tp, "g_sb")
            # H in T-major [16,256] for the stats AllGather
            ph = psum.tile([16, 256], f32, tag="ps")
            for k in range(2):
                nc.tensor.matmul(ph, tp[:, k, :], w_sb["wtrans"][:, k, :],
                                 start=(k == 0), stop=(k == 1))
            h32 = work.tile([16, 256], f32, tag="h32")
            nc.vector.tensor_copy(h32, ph)
            # H in c-major [128, 2, 16] for the phase-10 lhsT (scaled later)
            hc_sb = cmajor_mm("wtrans", tp, "hc_sb")

            # ---- phase 6+7 fused: dm = x G, exp, P-major [128, NI, 64] ----
            dmxw = big.tile([128, NI, 64], bf16, tag="bigB", name="dmxw")
            nc.vector.memset(dmxw[:, :, 17:64], 0.0)
            nc.vector.memset(dmxw[:, :, 16:17], 1.0)
            # dmxw slot order interleaves the two point-cloud halves
            # (slot 2q = tile q, slot 2q+1 = tile NT2+q) so the phase-7.5
            # transposes read contiguous [128, 2, 64] blocks.
            for g in range(NG):
                pdm = psum.tile([128, 8, 16], f32, tag="ps")
                for i8 in range(8):
                    i = g * 8 + i8
                    qt, lc = divmod(i, NI // 6)
                    for k in range(2):
                        nc.tensor.matmul(pdm[:, i8, :],
                                         xs[qt][:, k, lc * 128:(lc + 1) * 128],
                                         g_sb[:, k, :], start=(k == 0), stop=(k == 1))
                s0 = 16 * g if g < NG // 2 else 16 * g - NI + 1
                db = dmxw[:, s0, 0:16]
                dst = bass.AP(db.tensor, db.offset, [db.ap[0], [128, 8], [1, 16]])
                nc.scalar.activation(dst, pdm, AF.Exp, bias=shiftv)
            zd = work.tile([128, NI], f32, tag="zd")
            nc.vector.reduce_sum(zd, dmxw[:, :, 0:16], axis=AX)
            nc.vector.reciprocal(zd, zd)
            nc.vector.tensor_mul(zd, zd, mask_sb)
            zdb = bass.AP(zd.tensor, zd.offset, list(zd.ap) + [[0, T]])
            nc.vector.tensor_mul(dmxw[:, :, 0:16], dmxw[:, :, 0:16], zdb)

            # ---- phase 8: M = dmx^T dmx (+ u via ones column), AllGather ----
            pmu = psacc.tile([16, 17], f32, tag="acc", name="pmu")
            for i in range(NI):
                nc.tensor.matmul(pmu, dmxw[:, i, 0:16], dmxw[:, i, 0:17],
                                 start=(i == 0), stop=(i == NI - 1))
            # consume the warm-up collective on the GpSimd queue ONLY —
            # the tile scheduler reorders within engine queues, and a
            # warmup-dependent Vector op would block the whole Vector
            # queue until the warm-up AllGather lands (~90us).
            wz = work.tile([16, 1], f32, tag="wz")
            nc.gpsimd.dma_start(wz, wcc_out[0:16, 0:1])
            wzz = work.tile([16, 1], f32, tag="wzz")
            nc.gpsimd.tensor_scalar_mul(wzz, wz, 0.0)
            ccin = work.tile([16, GW + 257], f32, tag="ccin")
            M17 = work.tile([16, 17], f32, tag="M17")
            nc.scalar.copy(M17, pmu)
            mrep = bass.AP(M17.tensor, M17.offset,
                           [M17.ap[0], [0, n_cores], [1, 16]])
            nc.vector.tensor_mul(ccin[:, 0:GW], mrep, mfm_sb)
            nc.gpsimd.tensor_add(ccin[:, GW:GW + 1], M17[:, 16:17], wzz)
            nc.vector.tensor_copy(ccin[:, GW + 1:GW + 257], h32)
            ccd_in = dramp.tile([16, GW + 257], f32)
            ccd_out = dramp.tile([GW, GW + 257], f32)
            nc.sync.dma_start(ccd_in, ccin)
            nc.gpsimd.collective_compute(
                "AllGather", ALU.bypass,
                replica_groups=[list(range(n_cores))],
                ins=[ccd_in.opt()], outs=[ccd_out.opt()],
            )

            # ---- phase 7.5 (overlaps the gather): transpose dmx ----
            # dmxT block q holds tile q at partitions 0:17 (16 dm rows +
            # the ones row) and tile NT2+q at partitions 64:81, so each
            # phase-10 half covers a CONTIGUOUS 512-point range and the
            # tail adds / DMA need no strided operands.
            dmxT = big.tile([128, NT2, 128], bf16, tag="bigA", name="dmxT")
            for q in range(NT2):
                ptd = psum.tile([128, 128], bf16, tag="ps")
                nc.tensor.transpose(ptd, dmxw[:, 2 * q:2 * q + 2, :], identb)
                if q % 2 == 0:
                    nc.scalar.copy(dmxT[:, q, :], ptd)
                else:
                    nc.vector.tensor_copy(dmxT[:, q, :], ptd)

            gth = work.tile([GW, GW + 257], f32, tag="gth")
            nc.sync.dma_start(gth, ccd_out)

            # ---- phase 9: global BN stats from gathered {M, u, H} ----
            pY = psum.tile([GW, 256], f32, tag="ps")
            nc.tensor.matmul(pY, gth[:, 0:GW], gth[:, GW + 1:GW + 257])
            yh = work.tile([GW, 256], f32, tag="yh")
            nc.vector.tensor_mul(yh, pY, gth[:, GW + 1:GW + 257])
            pq = psum.tile([1, 256], f32, tag="ps")
            nc.tensor.matmul(pq, onescol[0:GW, :], yh)
            ps_ = psum.tile([1, 256], f32, tag="ps")
            nc.tensor.matmul(ps_, gth[:, GW:GW + 1], gth[:, GW + 1:GW + 257])
            sq = work.tile([1, 512], f32, tag="sq")
            nc.scalar.copy(sq[:, 0:256], ps_)
            nc.scalar.copy(sq[:, 256:512], pq)
            sqT = work.tile([128, 4], f32, tag="sqT")
            for h in range(4):
                pt = psum.tile([128, 1], f32, tag="ps")
                nc.tensor.transpose(pt, sq[:, h * 128:(h + 1) * 128], identf[0:1, 0:1])
                nc.scalar.copy(sqT[:, h:h + 1], pt)
            mean = work.tile([128, 2], f32, tag="mean")
            nc.vector.tensor_scalar_mul(mean, sqT[:, 0:2], 1.0 / n_total)
            ex2 = work.tile([128, 2], f32, tag="ex2")
            nc.vector.tensor_scalar_mul(ex2, sqT[:, 2:4], 1.0 / n_total)
            mm2 = work.tile([128, 2], f32, tag="mm2")
            nc.vector.tensor_mul(mm2, mean, mean)
            var = work.tile([128, 2], f32, tag="var")
            nc.vector.tensor_sub(var, ex2, mm2)
            sd = work.tile([128, 2], f32, tag="sd")
            nc.scalar.activation(sd, var, AF.Sqrt, bias=epsv)
            rstd = work.tile([128, 2], f32, tag="rstd")
            nc.vector.reciprocal(rstd, sd)
            a_sb = work.tile([128, 2], f32, tag="a_sb")
            nc.vector.tensor_mul(a_sb, gb_sb[:, 0:2], rstd)
            am = work.tile([128, 2], f32, tag="am")
            nc.vector.tensor_mul(am, a_sb, mean)
            b_sb = work.tile([128, 2], f32, tag="b_sb")
            nc.vector.tensor_sub(b_sb, gb_sb[:, 2:4], am)

            # ---- build the augmented lhsT: h2 rows = [H^T diag(a); b] ----
            hcb = work.tile([128, 2, 17], bf16, tag="hcb")
            for k in range(2):
                nc.vector.tensor_scalar_mul(hcb[:, k, 0:16], hc_sb[:, k, :],
                                            a_sb[:, k:k + 1])
                nc.vector.tensor_copy(hcb[:, k, 16:17], b_sb[:, k:k + 1])
            h2 = work.tile([128, 2, 128], bf16, tag="h2")
            for k in range(2):
                pth = psum.tile([17, 128], bf16, tag="ps")
                nc.tensor.transpose(pth, hcb[:, k, :], identb)
                nc.scalar.copy(h2[0:17, k, :], pth)
                nc.scalar.copy(h2[64:81, k, :], pth)

            # ---- phase 10: psum = a*xr + b (via 17-row matmuls), then
            # relu+residual split across ACT / DVE / Pool, DMA on Sync ----
            # Half-granular pipeline: each (jp,k,i2) half is one matmul
            # into its own 1-bank psum (bufs=4), one relu (psum->sbuf),
            # one residual add.  With the (q, NT2+q) transpose pairing
            # every half is a contiguous 512-point range, so relu, add,
            # and DMA all run on unit-stride operands.  Relus: ACT 48 /
            # DVE 24 (Pool cannot read PSUM); adds: DVE 40 / Pool 32.
            for jp in range(NJP):
                lo = (jp % 6) * 512
                ybuf = work.tile([128, 2, 2, 512], bf16, tag="ybuf", bufs=3)
                for k in range(2):
                    for i2 in range(2):
                        h = 2 * (2 * jp + k) + i2
                        base = 64 * i2
                        qt = jp // 6 + 3 * i2
                        pxh = psbig.tile([128, 512], f32, tag="pxr")
                        nc.tensor.matmul(pxh,
                                         h2[base:base + 17, k, :],
                                         dmxT[base:base + 17, 4 * jp:4 * jp + 4, :])
                        tt = work.tile([128, 512], bf16, tag="tt", bufs=6)
                        xw = xs[qt][:, k, lo:lo + 512]
                        if h % 7 == 3:
                            nc.vector.tensor_relu(tt, pxh)
                        else:
                            nc.scalar.activation(tt, pxh, AF.Relu)
                        if h % 2 == 0:
                            nc.gpsimd.tensor_add(ybuf[:, k, i2, :], tt, xw)
                        else:
                            nc.vector.tensor_add(ybuf[:, k, i2, :], tt, xw)
                for i2 in range(2):
                    off = i2 * NT2 * 128 + 512 * jp
                    nc.sync.dma_start(youtr[:, :, off:off + 512],
                                      ybuf[:, :, i2, :])

    nc.compile()
    return nc


def _prep_core(xc, P_pad, b, n_cores):
    bf = ml_dtypes.bfloat16
    cnt = xc.shape[0]
    NI = P_pad // 128
    xT = np.zeros((C, P_pad), dtype=bf)
    xT[:, :cnt] = xc.T.astype(bf)
    # P-major tiled layout [128, NI, C]: row p holds points i*128+p
    xp = np.zeros((NI * 128, C), dtype=bf)
    xp[:cnt] = xc.astype(bf)
    xp = np.ascontiguousarray(xp.reshape(NI, 128, C).transpose(1, 0, 2))
    idx = np.arange(P_pad).reshape(NI, 128).T  # [p, i] -> point index
    mask = (idx < cnt).astype(np.float32)
    # dmxw slot order: slot 2q = tile q, slot 2q+1 = tile NI//2+q
    perm = np.empty(NI, dtype=np.int64)
    perm[0::2] = np.arange(NI // 2)
    perm[1::2] = np.arange(NI // 2) + NI // 2
    mask = np.ascontiguousarray(mask[:, perm])
    mfm = np.zeros((16, 16 * n_cores), dtype=np.float32)
    mfm[:, b * 16:(b + 1) * 16] = 1.0
    npadv = np.full((1, 16), float(P_pad - cnt) * np.exp(-SHIFT), dtype=np.float32)
    return {"xT": xT, "xp": xp, "maskpm": mask, "mfm": mfm, "npadv": npadv}


def make_in_maps(x_f, counts, offs, P_pad, n_cores, Wq, Wk, Wp, Wv, Wke, Wqe,
                 Wemb, Wt, Wtrans, bn_gamma, bn_beta):
    bf = ml_dtypes.bfloat16
    g2 = np.asarray(bn_gamma, np.float32).reshape(2, 128).T
    b2 = np.asarray(bn_beta, np.float32).reshape(2, 128).T
    shared = {
        "wk": np.ascontiguousarray(Wk).astype(bf),
        "wq": np.ascontiguousarray(Wq).astype(bf),
        "wvT": np.ascontiguousarray(np.asarray(Wv).T).astype(bf),
        "wkeT": np.ascontiguousarray(np.asarray(Wke).T).astype(bf),
        "wqeT": np.ascontiguousarray(np.asarray(Wqe).T).astype(bf),
        "wembT": np.ascontiguousarray(np.asarray(Wemb).T).astype(bf),
        "wtT": np.ascontiguousarray(np.asarray(Wt).T).astype(bf),
        "wpT": np.ascontiguousarray(np.asarray(Wp).T).astype(bf),
        "wtrans": np.ascontiguousarray(Wtrans).astype(bf),
        "gb": np.concatenate([g2, b2], axis=1),
        "identb": np.eye(128, dtype=bf),
        "identf": np.eye(128, dtype=np.float32),
        "onesrow": np.ones((1, 128), dtype=np.float32),
        "onescol": np.ones((128, 1), dtype=np.float32),
    }
    in_maps = []
    for b in range(n_cores):
        m = _prep_core(x_f[offs[b]:offs[b + 1]], P_pad, b, n_cores)
        m.update(shared)
        in_maps.append(m)
    return in_maps


def kernel(x_f, batch_ids, Wq, Wk, Wp, Wv, Wke, Wqe, Wemb, Wt, Wtrans,
           bn_gamma, bn_beta):
    from concourse.bass_utils import run_bass_kernel_spmd

    x_f = np.asarray(x_f, dtype=np.float32)
    batch_ids = np.asarray(batch_ids)
    n_total = x_f.shape[0]
    counts = np.bincount(batch_ids, minlength=N_CORES)
    offs = np.concatenate([[0], np.cumsum(counts)])
    P_pad = int(-(-counts.max() // 6144) * 6144)

    key = (P_pad, N_CORES, n_total)
    if key not in _cache:
        _cache[key] = _build(P_pad, N_CORES, n_total)
    nc = _cache[key]

    in_maps = make_in_maps(x_f, counts, offs, P_pad, N_CORES, Wq, Wk, Wp, Wv,
                           Wke, Wqe, Wemb, Wt, Wtrans, bn_gamma, bn_beta)
    res = run_bass_kernel_spmd(nc, in_maps, list(range(N_CORES)))

    out = np.empty((n_total, C), dtype=np.float32)
    for b in range(N_CORES):
        yT = np.asarray(res.results[b]["yout"])  # [C, P_pad] bf16
        out[offs[b]:offs[b + 1]] = yT[:, :counts[b]].T.astype(np.float32)
    return out


# revision 29
# speedup vs baseline: 1.1188x; 1.0602x over previous
"""Trainium2 Bass kernel for the CSVT point-cloud token-attention block.

Strategy (8 NeuronCores, one point cloud per core):
  tokens = (Wq^T S) diag(1/z),  S = x^T e          (never materialize xq)
  dm     = x (Wp T_P)                              (never materialize xp)
  xr     = softmax(dm) (T_P^T Wtrans)              (never materialize df)
Global BatchNorm statistics travel as tiny per-cloud sufficient statistics
(M = dmx^T dmx, u = colsum dmx, H) via one small AllGather; a dummy warm-up
collective at kernel start absorbs the first-collective staging latency.

v2 changes vs the 207us baseline:
  - The warm-up consumer (wz/wzz) is emitted right before the real
    AllGather instead of at program start; the old placement put a
    warmup-dependent op at the head of the in-order Vector queue and
    stalled the whole machine ~45us mid-kernel.
  - Point-softmax logits are computed P-major directly (288 small
    matmuls with the x-tile as stationary weights) instead of T-major +
    144 PE transposes + psum copies; z comes from one DVE reduce plus a
    ones-column matmul.
  - BN scale/bias are folded into the PE: H is rebuilt c-major, scaled
    by a per-partition, augmented with a b-row, and matched with the
    ones-row already present in the transposed dmx; phase-10 psum is
    a*xr+b directly, so the tail is relu+add only, split 3 ways across
    ACT / DVE / Pool.  Output DMA issues from the Sync queue.
  - The transposed dmx keeps its ones row (17-row slabs at partition
    bases 0/64), so no separate bias/add pass is needed.
"""
import sys

sys.path.insert(0, "/opt/trn_rl_repo")

import numpy as np
import ml_dtypes

N_CORES = 8
C = 256
T = 16
EPS = 1e-5
SHIFT = 12.0

_cache = {}


def _build(P_pad, n_cores, n_total):
    import concourse.bass as bass
    import concourse.mybir as mybir
    import concourse.tile as tile
    from concourse import bacc

    bf16 = mybir.dt.bfloat16
    f32 = mybir.dt.float32
    AF = mybir.ActivationFunctionType
    AX = mybir.AxisListType.X
    ALU = mybir.AluOpType

    assert P_pad % 6144 == 0
    NI = P_pad // 128   # 128-point tiles
    NG = NI // 8        # 8-tile groups
    NT2 = NI // 2       # transpose blocks (2 tiles each)
    NB = NT2 // 2       # dmxT block columns (2 blocks per 128 partitions)
    QN = P_pad // 6     # points per x sixth
    NXP = NI // 16      # xpg groups
    NJP = P_pad // 1024 # output chunks per k-half
    GW = 16 * n_cores   # gathered stat rows

    nc = bacc.Bacc("TRN2", target_bir_lowering=False, debug=False)

    d_xT = nc.dram_tensor("xT", [C, P_pad], bf16, kind="ExternalInput").ap()
    # xp is host-pre-tiled P-major: [128, NI, C], row p holds points i*128+p
    d_xp = nc.dram_tensor("xp", [128, NI, C], bf16, kind="ExternalInput").ap()
    d_wk = nc.dram_tensor("wk", [C, T], bf16, kind="ExternalInput").ap()
    wnames = ["wq", "wvT", "wkeT", "wqeT", "wembT", "wtT", "wpT", "wtrans"]
    d_w = {n: nc.dram_tensor(n, [C, C], bf16, kind="ExternalInput").ap() for n in wnames}
    d_gb = nc.dram_tensor("gb", [128, 4], f32, kind="ExternalInput").ap()
    d_npad = nc.dram_tensor("npadv", [1, 16], f32, kind="ExternalInput").ap()
    d_mask = nc.dram_tensor("maskpm", [128, NI], f32, kind="ExternalInput").ap()
    d_mfm = nc.dram_tensor("mfm", [16, GW], f32, kind="ExternalInput").ap()
    d_identb = nc.dram_tensor("identb", [128, 128], bf16, kind="ExternalInput").ap()
    d_identf = nc.dram_tensor("identf", [128, 128], f32, kind="ExternalInput").ap()
    d_onesrow = nc.dram_tensor("onesrow", [1, 128], f32, kind="ExternalInput").ap()
    d_onescol = nc.dram_tensor("onescol", [128, 1], f32, kind="ExternalInput").ap()
    d_yout = nc.dram_tensor("yout", [C, P_pad], bf16, kind="ExternalOutput").ap()

    xTr = d_xT.rearrange("(k p) n -> p k n", p=128)
    youtr = d_yout.rearrange("(k p) n -> p k n", p=128)

    with tile.TileContext(nc) as tc:
        with (
            tc.tile_pool(name="const", bufs=1) as const,
            tc.tile_pool(name="xc", bufs=6) as xcp,
            tc.tile_pool(name="xpp", bufs=4) as xpp,
            tc.tile_pool(name="big", bufs=1) as big,
            tc.tile_pool(name="work", bufs=1) as work,
            tc.tile_pool(name="psum", bufs=3, space="PSUM") as psum,
            tc.tile_pool(name="psbig", bufs=4, space="PSUM") as psbig,
            tc.tile_pool(name="psacc", bufs=1, space="PSUM") as psacc,
            tc.tile_pool(name="dram", bufs=1, space="DRAM") as dramp,
        ):
            # ---- warm-up collective first (absorbs CC staging latency).
            # Its result is consumed much later, right before the real
            # AllGather, so nothing here blocks the main pipeline.
            ws = const.tile([16, 16], f32)
            nc.vector.memset(ws, 1.0)
            wcc_in = dramp.tile([16, 16], f32)
            wcc_out = dramp.tile([GW, 16], f32)
            nc.sync.dma_start(wcc_in, ws)
            nc.gpsimd.collective_compute(
                "AllGather", ALU.bypass,
                replica_groups=[list(range(n_cores))],
                ins=[wcc_in.opt()], outs=[wcc_out.opt()],
            )

            # ---- consts + weights FIRST on the Sync queue (~1.4 MB, ~4us).
            # NOT on the GpSimd queue: gpsimd-issued DMAs share a ring with
            # the collective and would sit behind the warm-up barrier. ----
            wk_sb = const.tile([128, 2, T], bf16)
            nc.sync.dma_start(wk_sb, d_wk.rearrange("(k p) t -> p k t", p=128))
            identb = const.tile([128, 128], bf16)
            nc.sync.dma_start(identb, d_identb)
            identf = const.tile([128, 128], f32)
            nc.sync.dma_start(identf, d_identf)
            onesrow = const.tile([1, 128], f32)
            nc.sync.dma_start(onesrow, d_onesrow)
            onescol = const.tile([128, 1], f32)
            nc.sync.dma_start(onescol, d_onescol)
            npad_sb = const.tile([1, 16], f32)
            nc.sync.dma_start(npad_sb, d_npad)
            mask_sb = const.tile([128, NI], f32)
            nc.sync.dma_start(mask_sb, d_mask)
            gb_sb = const.tile([128, 4], f32)
            nc.sync.dma_start(gb_sb, d_gb)
            mfm_sb = const.tile([16, GW], f32)
            nc.sync.dma_start(mfm_sb, d_mfm)
            w_sb = {}
            for n in wnames:
                w_sb[n] = const.tile([128, 2, C], bf16, tag=f"w_{n}", name=f"w_{n}")
                nc.sync.dma_start(w_sb[n], d_w[n].rearrange("(k p) c -> p k c", p=128))

            # ---- x stream on the Sync DMA queue: 6 sixths, 9 xpg ----
            xs = []
            for q in range(6):
                t = xcp.tile([128, 2, QN], bf16, tag="xc", name="xc")
                nc.sync.dma_start(t, xTr[:, :, q * QN:(q + 1) * QN])
                xs.append(t)
            xpgs = []
            for e in range(NXP):
                xpg = xpp.tile([128, 16, C], bf16, tag="xpg", name="xpg")
                nc.sync.dma_start(xpg, d_xp[:, e * 16:(e + 1) * 16, :])
                xpgs.append(xpg)
            epsv = const.tile([128, 1], f32)
            nc.vector.memset(epsv, EPS)
            shiftv = const.tile([128, 1], f32)
            nc.vector.memset(shiftv, -SHIFT)

            # ---- phase E: e = exp(x Wk - SHIFT), P-major [128, NI, 16] ----
            e_sb = big.tile([128, NI, T], bf16, tag="bigA", name="e_sb")
            for g in range(NG):
                pe = psum.tile([128, 8, 16], f32, tag="ps")
                for i8 in range(8):
                    i = g * 8 + i8
                    si, lc = divmod(i, NI // 6)
                    for k in range(2):
                        nc.tensor.matmul(pe[:, i8, :],
                                         xs[si][:, k, lc * 128:(lc + 1) * 128],
                                         wk_sb[:, k, :],
                                         start=(k == 0), stop=(k == 1))
                nc.scalar.activation(e_sb[:, g * 8:(g + 1) * 8, :], pe,
                                     AF.Exp, bias=shiftv)

            # ---- phase S: S^T = e^T x, accumulated over point tiles ----
            pS = psacc.tile([16, 256], f32, tag="acc")
            for e in range(NXP):
                for s in range(16):
                    i = e * 16 + s
                    nc.tensor.matmul(pS, e_sb[:, i, :], xpgs[e][:, s, :],
                                     start=(i == 0), stop=(i == NI - 1))
            sT = work.tile([16, 256], bf16, tag="sT")
            nc.scalar.copy(sT, pS)

            # ---- z = colsum e (pad-corrected), as a [1,16] row ----
            zpart = work.tile([128, 16], f32, tag="zpart")
            e_swp = bass.AP(e_sb.tensor, e_sb.offset,
                            [e_sb.ap[0], e_sb.ap[2], e_sb.ap[1]])
            nc.vector.reduce_sum(zpart, e_swp, axis=AX)
            zps = psum.tile([1, 16], f32, tag="ps")
            nc.tensor.matmul(zps, onescol, zpart)
            zrow = work.tile([1, 16], f32, tag="zrow")
            nc.scalar.copy(zrow, zps)
            zc = work.tile([1, 16], f32, tag="zc")
            nc.vector.tensor_sub(zc, zrow, npad_sb)  # npadv pre-scaled by e^-SHIFT
            zinv = work.tile([1, 16], f32, tag="zinv")
            nc.vector.reciprocal(zinv, zc)

            # ---- phase 5: tokens = (Wq^T S) diag(zinv) ----
            scm = work.tile([128, 2, 16], bf16, tag="scm")
            for k in range(2):
                pt = psum.tile([128, 16], bf16, tag="ps")
                nc.tensor.transpose(pt, sT[:, k * 128:(k + 1) * 128], identb[0:16, 0:16])
                nc.scalar.copy(scm[:, k, :], pt)
            pzb = psum.tile([128, 16], f32, tag="ps")
            nc.tensor.matmul(pzb, onesrow, zinv)
            zb = work.tile([128, 16], f32, tag="zb")
            nc.scalar.copy(zb, pzb)
            tok = work.tile([128, 2, 16], bf16, tag="tok")
            for ko in range(2):
                ptok = psum.tile([128, 16], f32, tag="ps")
                for ki in range(2):
                    nc.tensor.matmul(ptok, w_sb["wq"][:, ki, ko * 128:(ko + 1) * 128],
                                     scm[:, ki, :], start=(ki == 0), stop=(ki == 1))
                nc.vector.tensor_mul(tok[:, ko, :], ptok, zb)

            # ---- phase 5b: token self-attention ----
            def cmajor_mm(wname, rhs_tile, out_name, post=None):
                out = work.tile([128, 2, 16], bf16, tag=out_name, name=out_name)
                for ko in range(2):
                    p = psum.tile([128, 16], f32, tag="ps")
                    for ki in range(2):
                        nc.tensor.matmul(p, w_sb[wname][:, ki, ko * 128:(ko + 1) * 128],
                                         rhs_tile[:, ki, :], start=(ki == 0), stop=(ki == 1))
                    if post is None:
                        nc.scalar.copy(out[:, ko, :], p)
                    else:
                        post(out, ko, p)
                return out

            keys = cmajor_mm("wkeT", tok, "keys")
            qrs = cmajor_mm("wqeT", tok, "qrs")
            pv = psum.tile([16, 256], f32, tag="ps")
            for k in range(2):
                nc.tensor.matmul(pv, tok[:, k, :], w_sb["wvT"][:, k, :],
                                 start=(k == 0), stop=(k == 1))
            valsT = work.tile([16, 256], bf16, tag="valsT")
            nc.scalar.copy(valsT, pv)
            plg = psum.tile([16, 16], f32, tag="ps")
            for k in range(2):
                nc.tensor.matmul(plg, keys[:, k, :], qrs[:, k, :],
                                 start=(k == 0), stop=(k == 1))
            nmx2 = work.tile([16, 1], f32, tag="nmx2")
            nc.vector.reduce_max(nmx2, plg, axis=AX, negate=True)
            vtf = work.tile([16, 16], f32, tag="vtf")
            z2 = work.tile([16, 1], f32, tag="z2")
            nc.scalar.activation(vtf, plg, AF.Exp, bias=nmx2, accum_out=z2)
            z2i = work.tile([16, 1], f32, tag="z2i")
            nc.vector.reciprocal(z2i, z2)
            vt = work.tile([16, 16], bf16, tag="vt")
            nc.vector.tensor_scalar_mul(vt, vtf, z2i)
            pvtT = psum.tile([16, 16], bf16, tag="ps")
            nc.tensor.transpose(pvtT, vt, identb[0:16, 0:16])
            vtT = work.tile([16, 16], bf16, tag="vtT")
            nc.scalar.copy(vtT, pvtT)
            tm = work.tile([128, 2, 16], bf16, tag="tm")
            for ko in range(2):
                ptm = psum.tile([128, 16], f32, tag="ps")
                nc.tensor.matmul(ptm, valsT[:, ko * 128:(ko + 1) * 128], vtT)
                nc.scalar.copy(tm[:, ko, :], ptm)

            def add_tok(out, ko, p):
                nc.vector.tensor_add(out[:, ko, :], p, tok[:, ko, :])

            tout = cmajor_mm("wembT", tm, "tout", post=add_tok)
            tp = cmajor_mm("wtT", tout, "tp")
            g_sb = cmajor_mm("wpT", tp, "g_sb")
            # H in T-major [16,256] for the stats AllGather
            ph = psum.tile([16, 256], f32, tag="ps")
            for k in range(2):
                nc.tensor.matmul(ph, tp[:, k, :], w_sb["wtrans"][:, k, :],
                                 start=(k == 0), stop=(k == 1))
            h32 = work.tile([16, 256], f32, tag="h32")
            nc.vector.tensor_copy(h32, ph)
            # H in c-major [128, 2, 16] for the phase-10 lhsT (scaled later)
            hc_sb = cmajor_mm("wtrans", tp, "hc_sb")

            # ---- phase 6+7 fused: dm = x G, exp, P-major [128, NI, 64] ----
            dmxw = big.tile([128, NI, 64], bf16, tag="bigB", name="dmxw")
            nc.vector.memset(dmxw[:, :, 17:64], 0.0)
            nc.vector.memset(dmxw[:, :, 16:17], 1.0)
            # dmxw slot order interleaves the two point-cloud halves
            # (slot 2q = tile q, slot 2q+1 = tile NT2+q) so the phase-7.5
            # transposes read contiguous [128, 2, 64] blocks.
            for g in range(NG):
                pdm = psum.tile([128, 8, 16], f32, tag="ps")
                for i8 in range(8):
                    i = g * 8 + i8
                    qt, lc = divmod(i, NI // 6)
                    for k in range(2):
                        nc.tensor.matmul(pdm[:, i8, :],
                                         xs[qt][:, k, lc * 128:(lc + 1) * 128],
                                         g_sb[:, k, :], start=(k == 0), stop=(k == 1))
                s0 = 16 * g if g < NG // 2 else 16 * g - NI + 1
                db = dmxw[:, s0, 0:16]
                dst = bass.AP(db.tensor, db.offset, [db.ap[0], [128, 8], [1, 16]])
                nc.scalar.activation(dst, pdm, AF.Exp, bias=shiftv)
            zd = work.tile([128, NI], f32, tag="zd")
            nc.vector.reduce_sum(zd, dmxw[:, :, 0:16], axis=AX)
            nc.vector.reciprocal(zd, zd)
            nc.vector.tensor_mul(zd, zd, mask_sb)
            zdb = bass.AP(zd.tensor, zd.offset, list(zd.ap) + [[0, T]])
            nc.vector.tensor_mul(dmxw[:, :, 0:16], dmxw[:, :, 0:16], zdb)

            # ---- phase 8: M = dmx^T dmx (+ u via ones column), AllGather ----
            pmu = psacc.tile([16, 17], f32, tag="acc", name="pmu")
            for i in range(NI):
                nc.tensor.matmul(pmu, dmxw[:, i, 0:16], dmxw[:, i, 0:17],
                                 start=(i == 0), stop=(i == NI - 1))
            # consume the warm-up collective on the GpSimd queue ONLY —
            # the tile scheduler reorders within engine queues, and a
            # warmup-dependent Vector op would block the whole Vector
            # queue until the warm-up AllGather lands (~90us).
            wz = work.tile([16, 1], f32, tag="wz")
            nc.gpsimd.dma_start(wz, wcc_out[0:16, 0:1])
            wzz = work.tile([16, 1], f32, tag="wzz")
            nc.gpsimd.tensor_scalar_mul(wzz, wz, 0.0)
            ccin = work.tile([16, GW + 257], f32, tag="ccin")
            M17 = work.tile([16, 17], f32, tag="M17")
            nc.scalar.copy(M17, pmu)
            mrep = bass.AP(M17.tensor, M17.offset,
                           [M17.ap[0], [0, n_cores], [1, 16]])
            nc.vector.tensor_mul(ccin[:, 0:GW], mrep, mfm_sb)
            nc.gpsimd.tensor_add(ccin[:, GW:GW + 1], M17[:, 16:17], wzz)
            nc.vector.tensor_copy(ccin[:, GW + 1:GW + 257], h32)
            ccd_in = dramp.tile([16, GW + 257], f32)
            ccd_out = dramp.tile([GW, GW + 257], f32)
            nc.sync.dma_start(ccd_in, ccin)
            nc.gpsimd.collective_compute(
                "AllGather", ALU.bypass,
                replica_groups=[list(range(n_cores))],
                ins=[ccd_in.opt()], outs=[ccd_out.opt()],
            )

            # ---- phase 7.5 (overlaps the gather): transpose dmx ----
            # dmxT block q holds tile q at partitions 0:17 (16 dm rows +
            # the ones row) and tile NT2+q at partitions 64:81, so each
            # phase-10 half covers a CONTIGUOUS 512-point range and the
            # tail adds / DMA need no strided operands.
            dmxT = big.tile([128, NT2, 128], bf16, tag="bigA", name="dmxT")
            for q in range(NT2):
                ptd = psum.tile([128, 128], bf16, tag="ps")
                nc.tensor.transpose(ptd, dmxw[:, 2 * q:2 * q + 2, :], identb)
                if q % 2 == 0:
                    nc.scalar.copy(dmxT[:, q, :], ptd)
                else:
                    nc.vector.tensor_copy(dmxT[:, q, :], ptd)

            gth = work.tile([GW, GW + 257], f32, tag="gth")
            nc.sync.dma_start(gth, ccd_out)

            # ---- phase 9: global BN stats from gathered {M, u, H} ----
            pY = psum.tile([GW, 256], f32, tag="ps")
            nc.tensor.matmul(pY, gth[:, 0:GW], gth[:, GW + 1:GW + 257])
            yh = work.tile([GW, 256], f32, tag="yh")
            nc.vector.tensor_mul(yh, pY, gth[:, GW + 1:GW + 257])
            pq = psum.tile([1, 256], f32, tag="ps")
            nc.tensor.matmul(pq, onescol[0:GW, :], yh)
            ps_ = psum.tile([1, 256], f32, tag="ps")
            nc.tensor.matmul(ps_, gth[:, GW:GW + 1], gth[:, GW + 1:GW + 257])
            sq = work.tile([1, 512], f32, tag="sq")
            nc.scalar.copy(sq[:, 0:256], ps_)
            nc.scalar.copy(sq[:, 256:512], pq)
            sqT = work.tile([128, 4], f32, tag="sqT")
            for h in range(4):
                pt = psum.tile([128, 1], f32, tag="ps")
                nc.tensor.transpose(pt, sq[:, h * 128:(h + 1) * 128], identf[0:1, 0:1])
                nc.scalar.copy(sqT[:, h:h + 1], pt)
            mean = work.tile([128, 2], f32, tag="mean")
            nc.vector.tensor_scalar_mul(mean, sqT[:, 0:2], 1.0 / n_total)
            ex2 = work.tile([128, 2], f32, tag="ex2")
            nc.vector.tensor_scalar_mul(ex2, sqT[:, 2:4], 1.0 / n_total)
            mm2 = work.tile([128, 2], f32, tag="mm2")
            nc.vector.tensor_mul(mm2, mean, mean)
            var = work.tile([128, 2], f32, tag="var")
            nc.vector.tensor_sub(var, ex2, mm2)
            sd = work.tile([128, 2], f32, tag="sd")
            nc.scalar.activation(sd, var, AF.Sqrt, bias=epsv)
            rstd = work.tile([128, 2], f32, tag="rstd")
            nc.vector.reciprocal(rstd, sd)
            a_sb = work.tile([128, 2], f32, tag="a_sb")
            nc.vector.tensor_mul(a_sb, gb_sb[:, 0:2], rstd)
            am = work.tile([128, 2], f32, tag="am")
            nc.vector.tensor_mul(am, a_sb, mean)
            b_sb = work.tile([128, 2], f32, tag="b_sb")
            nc.vector.tensor_sub(b_sb, gb_sb[:, 2:4], am)

            # ---- build the augmented lhsT: h2 rows = [H^T diag(a); b] ----
            hcb = work.tile([128, 2, 17], bf16, tag="hcb")
            for k in range(2):
                nc.vector.tensor_scalar_mul(hcb[:, k, 0:16], hc_sb[:, k, :],
                                            a_sb[:, k:k + 1])
                nc.vector.tensor_copy(hcb[:, k, 16:17], b_sb[:, k:k + 1])
            h2 = work.tile([128, 2, 128], bf16, tag="h2")
            for k in range(2):
                pth = psum.tile([17, 128], bf16, tag="ps")
                nc.tensor.transpose(pth, hcb[:, k, :], identb)
                nc.scalar.copy(h2[0:17, k, :], pth)
                nc.scalar.copy(h2[64:81, k, :], pth)

            # ---- phase 10: psum = a*xr + b (via 17-row matmuls), then
            # relu+residual split across ACT / DVE / Pool, DMA on Sync ----
            # Half-granular pipeline: each (jp,k,i2) half is one matmul
            # into its own 1-bank psum (bufs=4), one relu (psum->sbuf),
            # one residual add.  With the (q, NT2+q) transpose pairing
            # every half is a contiguous 512-point range, so relu, add,
            # and DMA all run on unit-stride operands.  Relus: ACT 48 /
            # DVE 24 (Pool cannot read PSUM); adds: DVE 40 / Pool 32.
            for jp in range(NJP):
                lo = (jp % 6) * 512
                ybuf = work.tile([128, 2, 2, 512], bf16, tag="ybuf", bufs=4)
                for k in range(2):
                    for i2 in range(2):
                        h = 2 * (2 * jp + k) + i2
                        base = 64 * i2
                        qt = jp // 6 + 3 * i2
                        pxh = psbig.tile([128, 512], f32, tag="pxr")
                        nc.tensor.matmul(pxh,
                                         h2[base:base + 17, k, :],
                                         dmxT[base:base + 17, 4 * jp:4 * jp + 4, :])
                        pool_add = (h % 2 == 0)
                        tt = work.tile([128, 512], bf16,
                                       tag="ttP" if pool_add else "ttD",
                                       name="tt", bufs=4)
                        xw = xs[qt][:, k, lo:lo + 512]
                        if h % 7 == 3:
                            nc.vector.tensor_relu(tt, pxh)
                        else:
                            nc.scalar.activation(tt, pxh, AF.Relu)
                        if pool_add:
                            nc.gpsimd.tensor_add(ybuf[:, k, i2, :], tt, xw)
                        else:
                            nc.vector.tensor_add(ybuf[:, k, i2, :], tt, xw)
                for i2 in range(2):
                    off = i2 * NT2 * 128 + 512 * jp
                    nc.sync.dma_start(youtr[:, :, off:off + 512],
                                      ybuf[:, :, i2, :])

    nc.compile()
    return nc


def _prep_core(xc, P_pad, b, n_cores):
    bf = ml_dtypes.bfloat16
    cnt = xc.shape[0]
    NI = P_pad // 128
    xT = np.zeros((C, P_pad), dtype=bf)
    xT[:, :cnt] = xc.T.astype(bf)
    # P-major tiled layout [128, NI, C]: row p holds points i*128+p
    xp = np.zeros((NI * 128, C), dtype=bf)
    xp[:cnt] = xc.astype(bf)
    xp = np.ascontiguousarray(xp.reshape(NI, 128, C).transpose(1, 0, 2))
    idx = np.arange(P_pad).reshape(NI, 128).T  # [p, i] -> point index
    mask = (idx < cnt).astype(np.float32)
    # dmxw slot order: slot 2q = tile q, slot 2q+1 = tile NI//2+q
    perm = np.empty(NI, dtype=np.int64)
    perm[0::2] = np.arange(NI // 2)
    perm[1::2] = np.arange(NI // 2) + NI // 2
    mask = np.ascontiguousarray(mask[:, perm])
    mfm = np.zeros((16, 16 * n_cores), dtype=np.float32)
    mfm[:, b * 16:(b + 1) * 16] = 1.0
    npadv = np.full((1, 16), float(P_pad - cnt) * np.exp(-SHIFT), dtype=np.float32)
    return {"xT": xT, "xp": xp, "maskpm": mask, "mfm": mfm, "npadv": npadv}


def make_in_maps(x_f, counts, offs, P_pad, n_cores, Wq, Wk, Wp, Wv, Wke, Wqe,
                 Wemb, Wt, Wtrans, bn_gamma, bn_beta):
    bf = ml_dtypes.bfloat16
    g2 = np.asarray(bn_gamma, np.float32).reshape(2, 128).T
    b2 = np.asarray(bn_beta, np.float32).reshape(2, 128).T
    shared = {
        "wk": np.ascontiguousarray(Wk).astype(bf),
        "wq": np.ascontiguousarray(Wq).astype(bf),
        "wvT": np.ascontiguousarray(np.asarray(Wv).T).astype(bf),
        "wkeT": np.ascontiguousarray(np.asarray(Wke).T).astype(bf),
        "wqeT": np.ascontiguousarray(np.asarray(Wqe).T).astype(bf),
        "wembT": np.ascontiguousarray(np.asarray(Wemb).T).astype(bf),
        "wtT": np.ascontiguousarray(np.asarray(Wt).T).astype(bf),
        "wpT": np.ascontiguousarray(np.asarray(Wp).T).astype(bf),
        "wtrans": np.ascontiguousarray(Wtrans).astype(bf),
        "gb": np.concatenate([g2, b2], axis=1),
        "identb": np.eye(128, dtype=bf),
        "identf": np.eye(128, dtype=np.float32),
        "onesrow": np.ones((1, 128), dtype=np.float32),
        "onescol": np.ones((128, 1), dtype=np.float32),
    }
    in_maps = []
    for b in range(n_cores):
        m = _prep_core(x_f[offs[b]:offs[b + 1]], P_pad, b, n_cores)
        m.update(shared)
        in_maps.append(m)
    return in_maps


def kernel(x_f, batch_ids, Wq, Wk, Wp, Wv, Wke, Wqe, Wemb, Wt, Wtrans,
           bn_gamma, bn_beta):
    from concourse.bass_utils import run_bass_kernel_spmd

    x_f = np.asarray(x_f, dtype=np.float32)
    batch_ids = np.asarray(batch_ids)
    n_total = x_f.shape[0]
    counts = np.bincount(batch_ids, minlength=N_CORES)
    offs = np.concatenate([[0], np.cumsum(counts)])
    P_pad = int(-(-counts.max() // 6144) * 6144)

    key = (P_pad, N_CORES, n_total)
    if key not in _cache:
        _cache[key] = _build(P_pad, N_CORES, n_total)
    nc = _cache[key]

    in_maps = make_in_maps(x_f, counts, offs, P_pad, N_CORES, Wq, Wk, Wp, Wv,
                           Wke, Wqe, Wemb, Wt, Wtrans, bn_gamma, bn_beta)
    res = run_bass_kernel_spmd(nc, in_maps, list(range(N_CORES)))

    out = np.empty((n_total, C), dtype=np.float32)
    for b in range(N_CORES):
        yT = np.asarray(res.results[b]["yout"])  # [C, P_pad] bf16
        out[offs[b]:offs[b + 1]] = yT[:, :counts[b]].T.astype(np.float32)
    return out
